# revision 19
# baseline (speedup 1.0000x reference)
"""Trainium2 Bass kernel for nn_BackBone (LSTM backbone + fc + outer-product head).

Data-parallel over batch across 8 NeuronCores. Per core (b_loc rows), v4:
  - history transposed + cast to fp16 on the HOST: xt[D+1, T, B] with a
    constant-1.0 feature row appended so the gate bias rides the projection
    matmul (no PE transposes, no SWDGE cast, half the input HBM traffic)
  - two 512-col batch chains. Tile WAR deps are TILE-granular, so the gate
    PSUM is split per READER: Pif [128,2,512] (read only by the merged
    sigmoid(i,f)), Pg (tanh), Po (sigmoid) — a projection matmul for gate X
    then only waits on gate X's own activation from the previous step.
  - PE stream per step: recA(i,f,g,o), recB, projA(t+1) in reader order
    (if x3k, g x3k, o x3k), projB(t+1) — zero-stall steady state
  - head einsum m-major (out[b, l, m, n]; host un-permutes): y2 half on
    GpSimd (idle engine; broadcast muls run FASTER there than on DVE),
    y1 tail split GpSimd (broadcast) + DVE (per-(l,m) tensor_scalar, 2x
    mode) + ACT (per-(l,m) scalar.mul)
  - output fp16 as two [b, L, 3, 128] tensors; host concatenates/permutes/
    casts; loads on sync HWDGE, stores on gpsimd SWDGE
"""
import numpy as np

import concourse.bacc as bacc
import concourse.mybir as mybir
import concourse.tile as tile
from concourse import bass_utils

F32 = mybir.dt.float32
F16 = mybir.dt.float16
AF = mybir.ActivationFunctionType

T = 20
D = 340
DP = D + 1               # +1 constant feature row carrying the gate bias
H = 128
E = 32
L = 10
M3 = 3
DCH = [(0, 128), (128, 256), (256, DP)]   # contraction chunks of DP
N_CORES = 8


def build_program(b_loc: int):
    assert b_loc % 256 == 0
    NJ = b_loc // 128
    CW = b_loc // 2               # chain width (<= 512)
    assert CW <= 512
    NCB = 2
    if T == 20:
        TGR = [(0, 1), (1, 2)] + [(t, t + 2) for t in range(2, 14, 2)] \
            + [(14, 17), (17, 20)]
    else:
        TGR = [(0, T)]

    nc = bacc.Bacc("TRN2", target_bir_lowering=False, debug=False)
    xt_d = nc.dram_tensor("xt", (DP, T, b_loc), F16, kind="ExternalInput").ap()
    cnt_d = nc.dram_tensor("cn_t", (E, b_loc), F16, kind="ExternalInput").ap()
    pref_d = nc.dram_tensor("pref_g", (128, NJ, L, M3), F16,
                            kind="ExternalInput").ap()
    pref32_d = nc.dram_tensor("pref_g32", (128, NJ, L, M3), F32,
                              kind="ExternalInput").ap()
    wih_d = nc.dram_tensor("w_ih4", (DP, 4 * H), F16, kind="ExternalInput").ap()
    whh_d = nc.dram_tensor("w_hh_t", (H, 4 * H), F16, kind="ExternalInput").ap()
    fcw_d = nc.dram_tensor("fc_w_t", (E, H), F16, kind="ExternalInput").ap()
    fcb_d = nc.dram_tensor("fc_b_row", (1, H), F16, kind="ExternalInput").ap()
    ones_d = nc.dram_tensor("ones_row", (1, 128), F16, kind="ExternalInput").ap()
    oy1 = nc.dram_tensor("out_y1", (b_loc, L, M3, 128), F16,
                         kind="ExternalOutput").ap()
    oy2 = nc.dram_tensor("out_y2", (b_loc, L, M3, 128), F16,
                         kind="ExternalOutput").ap()

    with tile.TileContext(nc) as tc:
        with tc.tile_pool(name="wpool", bufs=1) as wpool, \
             tc.tile_pool(name="main", bufs=1) as pool, \
             tc.tile_pool(name="psum", bufs=1, space="PSUM") as pspool:

            # ---- weights / constants ----
            wih_t = []
            for k, (c0, c1) in enumerate(DCH):
                wt_ = wpool.tile([c1 - c0, 4 * H], F16, name=f"wih{k}")
                nc.sync.dma_start(wt_[:], wih_d[c0:c1, :])
                wih_t.append(wt_)
            whh_t = wpool.tile([H, 4 * H], F16, name="whh_t")
            nc.sync.dma_start(whh_t[:], whh_d)
            cnt_t = wpool.tile([E, b_loc], F16, name="cnt_t")
            nc.sync.dma_start(cnt_t[:], cnt_d)
            fcw_t = wpool.tile([E, H], F16, name="fcw_t")
            nc.sync.dma_start(fcw_t[:], fcw_d)
            fcb_t = wpool.tile([1, H], F16, name="fcb_t")
            nc.sync.dma_start(fcb_t[:], fcb_d)
            ones_t = wpool.tile([1, 128], F16, name="ones_t")
            nc.sync.dma_start(ones_t[:], ones_d)
            pf_t = wpool.tile([128, NJ, L, M3], F16, name="pf_t")
            nc.sync.dma_start(pf_t[:], pref_d)
            pf32_t = wpool.tile([128, NJ, L, M3], F32, name="pf32_t")
            nc.sync.dma_start(pf32_t[:], pref32_d)

            # ---- persistent fp16 xT tiles, loaded in t-groups ----
            xt_tiles = []
            for k, (c0, c1) in enumerate(DCH):
                xt_tiles.append(
                    pool.tile([c1 - c0, T, b_loc], F16, name=f"xt{k}",
                              tag=f"xt{k}"))
            for (t0, t1) in TGR:
                for k, (c0, c1) in enumerate(DCH):
                    nc.sync.dma_start(xt_tiles[k][:, t0:t1, :],
                                      xt_d[c0:c1, t0:t1, :])

            # ---- PSUM per chain, split per ACT reader ----
            Pif, Pg, Po = [], [], []
            for cb in range(NCB):
                Pif.append(pspool.tile([128, 2, 512], F32, name=f"pif{cb}",
                                       tag=f"pif{cb}"))
                Pg.append(pspool.tile([128, 512], F32, name=f"pg{cb}",
                                      tag=f"pg{cb}"))
                Po.append(pspool.tile([128, 512], F32, name=f"po{cb}",
                                      tag=f"po{cb}"))

            def emit_einsum_bcast(j, y_half, odram, l0, nl, engine):
                """broadcast tensor_mul form (good on GpSimd)."""
                rows = j * 128
                ol = pool.tile([128, nl, M3, 128], F16, name="ol",
                               tag="outl", bufs=10)
                y_b = y_half[:, None, None, :].broadcast_to(
                    [128, nl, M3, 128])
                p_b = pf_t[:, j, l0:l0 + nl, :, None].broadcast_to(
                    [128, nl, M3, 128])
                engine.tensor_mul(ol[:], y_b, p_b)
                nc.gpsimd.dma_start(
                    odram[rows:rows + 128, l0:l0 + nl, :, :], ol[:])

            def emit_einsum_ts(j, y_half, odram, l0, nl, engine, use_act):
                """per-(l,m) form: DVE tensor_scalar (2x) or ACT mul."""
                rows = j * 128
                ol = pool.tile([128, nl, M3, 128], F16, name="ol",
                               tag="outl", bufs=10)
                for li in range(nl):
                    for m in range(M3):
                        sc = pf32_t[:, j, l0 + li, m:m + 1]
                        if use_act:
                            engine.mul(ol[:, li, m, :], y_half[:], sc)
                        else:
                            engine.tensor_scalar_mul(ol[:, li, m, :],
                                                     y_half[:], sc)
                nc.gpsimd.dma_start(
                    odram[rows:rows + 128, l0:l0 + nl, :, :], ol[:])

            store_jobs = []          # deferred y2 einsum+store emitters

            # ---- y2 head (prologue): borrows Pif[0] slices before t=0 ----
            for j in range(NJ):
                tgt = Pif[0][:, j // 4, (j % 4) * 128:(j % 4) * 128 + 128]
                nc.tensor.matmul(tgt, cnt_t[:, j * 128:(j + 1) * 128],
                                 fcw_t[:], start=True, stop=False)
                nc.tensor.matmul(tgt, ones_t[:], fcb_t[:],
                                 start=False, stop=True)
                yb = pool.tile([128, 128], F16, name="y2b", tag="y2b",
                               bufs=NJ)
                nc.scalar.activation(yb[:], tgt, AF.Relu)
                if j < NJ - 1:          # DVE tensor_scalar, 3l/2l bites
                    for l0, nl in ((0, 3), (3, 3), (6, 2), (8, 2)):
                        store_jobs.append(('ts', j, yb, oy2, l0, nl))
                else:                   # last j-tile: ACT in 1l bites
                    for l0 in range(L):
                        store_jobs.append(('act', j, yb, oy2, l0, 1))

            def emit_proj(t, cb, stop):
                cs = slice(cb * CW, (cb + 1) * CW)

                def mm(dst, g, k):
                    nc.tensor.matmul(
                        dst, wih_t[k][:, g * 128:(g + 1) * 128],
                        xt_tiles[k][:, t, cs],
                        start=(k == 0), stop=(stop and k == 2))
                for k in range(3):              # i, f pairs first
                    mm(Pif[cb][:, 0, 0:CW], 0, k)
                    mm(Pif[cb][:, 1, 0:CW], 1, k)
                for k in range(3):              # then g
                    mm(Pg[cb][:, 0:CW], 2, k)
                for k in range(3):              # then o
                    mm(Po[cb][:, 0:CW], 3, k)

            def emit_rec(cb, h_prev):
                for g, dst in ((0, Pif[cb][:, 0, 0:CW]),
                               (1, Pif[cb][:, 1, 0:CW]),
                               (2, Pg[cb][:, 0:CW]),
                               (3, Po[cb][:, 0:CW])):
                    nc.tensor.matmul(dst, whh_t[:, g * 128:(g + 1) * 128],
                                     h_prev[:], start=False, stop=True)

            def new_state(tag):
                return pool.tile([128, CW], F16, name=tag, tag=tag, bufs=2)

            h_prev = [None, None]
            c_prev = [None, None]

            # ---- prologue projections for t=0 ----
            emit_proj(0, 0, stop=True)
            emit_proj(0, 1, stop=True)

            # ---- recurrence ----
            job_i = 0
            for t in range(T):
                if t > 0:
                    emit_rec(0, h_prev[0])
                    emit_rec(1, h_prev[1])

                gif = [pool.tile([128, 2, CW], F16, name="gif",
                                 tag=f"gif{cb}", bufs=2) for cb in range(NCB)]
                gg = [new_state(f"gg{cb}") for cb in range(NCB)]
                go = [new_state(f"go{cb}") for cb in range(NCB)]
                c_t = ([new_state(f"c{cb}") for cb in range(NCB)]
                       if t > 0 else [None, None])
                tc_t = [new_state(f"tc{cb}") for cb in range(NCB)]
                h_t = [new_state(f"h{cb}") for cb in range(NCB)]
                t1 = ([new_state(f"t1{cb}") for cb in range(NCB)]
                      if t > 0 else [None, None])
                t2 = [new_state(f"t2{cb}") for cb in range(NCB)]

                for cb in range(NCB):
                    nc.scalar.activation(gif[cb][:], Pif[cb][:, :, 0:CW],
                                         AF.Sigmoid)
                    nc.scalar.activation(gg[cb][:], Pg[cb][:, 0:CW], AF.Tanh)
                    nc.scalar.activation(go[cb][:], Po[cb][:, 0:CW],
                                         AF.Sigmoid)
                    if t > 0:
                        nc.vector.tensor_mul(t1[cb][:], gif[cb][:, 1, :],
                                             c_prev[cb][:])
                    nc.vector.tensor_mul(t2[cb][:], gif[cb][:, 0, :],
                                         gg[cb][:])
                    if t > 0:
                        nc.vector.tensor_add(c_t[cb][:], t1[cb][:],
                                             t2[cb][:])
                    else:
                        c_t[cb] = t2[cb]
                # tanh(c) + h after both chains' gate ACTs are queued
                for cb in range(NCB):
                    nc.scalar.activation(tc_t[cb][:], c_t[cb][:], AF.Tanh)
                    nc.vector.tensor_mul(h_t[cb][:], go[cb][:], tc_t[cb][:])

                # PE: projections for t+1
                if t + 1 < T:
                    emit_proj(t + 1, 0, stop=False)
                    emit_proj(t + 1, 1, stop=False)

                h_prev = h_t
                c_prev = c_t

                # y2 einsum+stores through the recurrence.  DVE
                # tensor_scalar rides the dedicated SBUF port; GpSimd bites
                # are kept <=2l so their exclusive hold of the shared
                # DVE/GpSimd port pair can't delay the h-chain by much.
                if t >= 2:
                    if t < T - 1:
                        take, n_ts, seen_act = [], 0, False
                        for job in store_jobs:
                            if job[0] == 'ts' and n_ts < 2:
                                take.append(job); n_ts += 1
                            elif job[0] == 'act' and not seen_act:
                                take.append(job); seen_act = True
                        for job in take:
                            store_jobs.remove(job)
                            kind, j, yb, od, l0, nl = job
                            if kind == 'ts':
                                emit_einsum_ts(j, yb, od, l0, nl,
                                               nc.vector, False)
                            else:
                                emit_einsum_ts(j, yb, od, l0, nl,
                                               nc.scalar, True)
                    else:
                        for job in store_jobs:
                            kind, j, yb, od, l0, nl = job
                            eng = nc.vector if kind == 'ts' else nc.scalar
                            emit_einsum_ts(j, yb, od, l0, nl, eng,
                                           kind != 'ts')
                        store_jobs.clear()

            # ---- tail: y1 half ----
            NJH = NJ // 2
            y1b = []
            for j in range(NJ):
                y1 = pool.tile([128, 128], F16, name="y1b", tag="y1b",
                               bufs=NJ)
                src = h_prev[j // NJH][:, (j % NJH) * 128:(j % NJH) * 128 + 128]
                nc.sync.dma_start_transpose(y1[:], src)
                y1b.append(y1)
            # split 16 (j, l-chunk) jobs: gpsimd 6, DVE 6, ACT 4
            jobs = [(j, l0) for j in range(NJ) for l0 in range(0, L, 5)]
            for idx, (j, l0) in enumerate(jobs):
                r = idx % 16
                if r in (0, 2, 4, 7, 9, 11, 13, 15):
                    emit_einsum_bcast(j, y1b[j], oy1, l0, 5, nc.gpsimd)
                elif r in (1, 6, 10, 14):
                    emit_einsum_ts(j, y1b[j], oy1, l0, 5, nc.scalar, True)
                else:
                    emit_einsum_ts(j, y1b[j], oy1, l0, 5, nc.vector, False)

    nc.compile()
    return nc


def prep_in_maps(inputs, n_cores: int, b_loc: int):
    history = np.asarray(inputs["history"], np.float32)
    cluster = np.asarray(inputs["cluster_num"], np.float32)
    pref = np.asarray(inputs["pref"], np.float32)
    w_ih = np.asarray(inputs["W_ih"], np.float32)
    w_hh = np.asarray(inputs["W_hh"], np.float32)
    b_ih = np.asarray(inputs["b_ih"], np.float32)
    b_hh = np.asarray(inputs["b_hh"], np.float32)
    fc_w = np.asarray(inputs["fc_w"], np.float32)
    fc_b = np.asarray(inputs["fc_b"], np.float32)

    NJ = b_loc // 128
    w_ih4 = np.concatenate(
        [w_ih.T, (b_ih + b_hh).reshape(1, 4 * H)], axis=0)  # [341, 512]
    shared = {
        "w_ih4": np.ascontiguousarray(w_ih4.astype(np.float16)),
        "w_hh_t": np.ascontiguousarray(w_hh.T.astype(np.float16)),
        "fc_w_t": np.ascontiguousarray(fc_w.T.astype(np.float16)),
        "fc_b_row": np.ascontiguousarray(fc_b.reshape(1, H).astype(np.float16)),
        "ones_row": np.ones((1, 128), np.float16),
    }
    in_maps = []
    for c in range(n_cores):
        r0, r1 = c * b_loc, (c + 1) * b_loc
        hist16 = history[r0:r1].reshape(b_loc, T, D).astype(np.float16)
        xt = np.empty((DP, T, b_loc), np.float16)
        xt[:D] = hist16.transpose(2, 1, 0)
        xt[D] = 1.0
        pref16 = pref[r0:r1].reshape(NJ, 128, L, M3).astype(np.float16)
        pg = np.ascontiguousarray(pref16.transpose(1, 0, 2, 3))
        in_maps.append({
            "xt": xt,
            "cn_t": np.ascontiguousarray(
                cluster[r0:r1].T.astype(np.float16)),
            "pref_g": pg,
            "pref_g32": pg.astype(np.float32),
            **shared,
        })
    return in_maps


def run(inputs, n_cores: int = N_CORES, trace: bool = False):
    B = np.asarray(inputs["history"]).shape[0]
    b_loc = B // n_cores
    nc = build_program(b_loc)
    in_maps = prep_in_maps(inputs, n_cores, b_loc)
    res = bass_utils.run_bass_kernel_spmd(
        nc, in_maps, core_ids=list(range(n_cores)), trace=trace)
    outs = []
    for c in range(n_cores):
        y1 = res.results[c]["out_y1"].astype(np.float32)
        y2 = res.results[c]["out_y2"].astype(np.float32)
        o = np.concatenate([y1, y2], axis=3)         # [b, L, M3, 256]
        outs.append(o.transpose(0, 1, 3, 2).reshape(b_loc, L, 256 * M3))
    return np.concatenate(outs, axis=0), res


def kernel(**inputs) -> np.ndarray:
    out, _ = run(inputs, N_CORES)
    return out


# revision 20
# speedup vs baseline: 1.0107x; 1.0107x over previous
"""Trainium2 Bass kernel for nn_BackBone (LSTM backbone + fc + outer-product head).

Data-parallel over batch across 8 NeuronCores. Per core (b_loc rows), v4:
  - history transposed + cast to fp16 on the HOST: xt[D+1, T, B] with a
    constant-1.0 feature row appended so the gate bias rides the projection
    matmul (no PE transposes, no SWDGE cast, half the input HBM traffic)
  - two 512-col batch chains. Tile WAR deps are TILE-granular, so the gate
    PSUM is split per READER: Pif [128,2,512] (read only by the merged
    sigmoid(i,f)), Pg (tanh), Po (sigmoid) — a projection matmul for gate X
    then only waits on gate X's own activation from the previous step.
  - PE stream per step: recA(i,f,g,o), recB, projA(t+1) in reader order
    (if x3k, g x3k, o x3k), projB(t+1) — zero-stall steady state
  - head einsum m-major (out[b, l, m, n]; host un-permutes): y2 half on
    GpSimd (idle engine; broadcast muls run FASTER there than on DVE),
    y1 tail split GpSimd (broadcast) + DVE (per-(l,m) tensor_scalar, 2x
    mode) + ACT (per-(l,m) scalar.mul)
  - output fp16 as two [b, L, 3, 128] tensors; host concatenates/permutes/
    casts; loads on sync HWDGE, stores on gpsimd SWDGE
"""
import numpy as np

import concourse.bacc as bacc
import concourse.mybir as mybir
import concourse.tile as tile
from concourse import bass_utils

F32 = mybir.dt.float32
F16 = mybir.dt.float16
AF = mybir.ActivationFunctionType

T = 20
D = 340
DP = D + 1               # +1 constant feature row carrying the gate bias
H = 128
E = 32
L = 10
M3 = 3
DCH = [(0, 128), (128, 256), (256, DP)]   # contraction chunks of DP
N_CORES = 8


def build_program(b_loc: int):
    assert b_loc % 256 == 0
    NJ = b_loc // 128
    CW = b_loc // 2               # chain width (<= 512)
    assert CW <= 512
    NCB = 2
    if T == 20:
        TGR = [(0, 1), (1, 2)] + [(t, t + 2) for t in range(2, 14, 2)] \
            + [(14, 17), (17, 20)]
    else:
        TGR = [(0, T)]

    nc = bacc.Bacc("TRN2", target_bir_lowering=False, debug=False)
    xt_d = nc.dram_tensor("xt", (DP, T, b_loc), F16, kind="ExternalInput").ap()
    cnt_d = nc.dram_tensor("cn_t", (E, b_loc), F16, kind="ExternalInput").ap()
    pref_d = nc.dram_tensor("pref_g", (128, NJ, L, M3), F16,
                            kind="ExternalInput").ap()
    pref32_d = nc.dram_tensor("pref_g32", (128, NJ, L, M3), F32,
                              kind="ExternalInput").ap()
    wih_d = nc.dram_tensor("w_ih4", (DP, 4 * H), F16, kind="ExternalInput").ap()
    whh_d = nc.dram_tensor("w_hh_t", (H, 4 * H), F16, kind="ExternalInput").ap()
    fcw_d = nc.dram_tensor("fc_w_t", (E, H), F16, kind="ExternalInput").ap()
    fcb_d = nc.dram_tensor("fc_b_row", (1, H), F16, kind="ExternalInput").ap()
    ones_d = nc.dram_tensor("ones_row", (1, 128), F16, kind="ExternalInput").ap()
    oy1 = nc.dram_tensor("out_y1", (b_loc, L, M3, 128), F16,
                         kind="ExternalOutput").ap()
    oy2 = nc.dram_tensor("out_y2", (b_loc, L, M3, 128), F16,
                         kind="ExternalOutput").ap()

    with tile.TileContext(nc) as tc:
        with tc.tile_pool(name="wpool", bufs=1) as wpool, \
             tc.tile_pool(name="main", bufs=1) as pool, \
             tc.tile_pool(name="psum", bufs=1, space="PSUM") as pspool:

            # ---- weights / constants ----
            wih_t = []
            for k, (c0, c1) in enumerate(DCH):
                wt_ = wpool.tile([c1 - c0, 4 * H], F16, name=f"wih{k}")
                nc.sync.dma_start(wt_[:], wih_d[c0:c1, :])
                wih_t.append(wt_)
            whh_t = wpool.tile([H, 4 * H], F16, name="whh_t")
            nc.sync.dma_start(whh_t[:], whh_d)
            cnt_t = wpool.tile([E, b_loc], F16, name="cnt_t")
            nc.sync.dma_start(cnt_t[:], cnt_d)
            fcw_t = wpool.tile([E, H], F16, name="fcw_t")
            nc.sync.dma_start(fcw_t[:], fcw_d)
            fcb_t = wpool.tile([1, H], F16, name="fcb_t")
            nc.sync.dma_start(fcb_t[:], fcb_d)
            ones_t = wpool.tile([1, 128], F16, name="ones_t")
            nc.sync.dma_start(ones_t[:], ones_d)
            pf_t = wpool.tile([128, NJ, L, M3], F16, name="pf_t")
            nc.sync.dma_start(pf_t[:], pref_d)
            pf32_t = wpool.tile([128, NJ, L, M3], F32, name="pf32_t")
            nc.sync.dma_start(pf32_t[:], pref32_d)

            # ---- persistent fp16 xT tiles, loaded in t-groups ----
            xt_tiles = []
            for k, (c0, c1) in enumerate(DCH):
                xt_tiles.append(
                    pool.tile([c1 - c0, T, b_loc], F16, name=f"xt{k}",
                              tag=f"xt{k}"))
            for (t0, t1) in TGR:
                for k, (c0, c1) in enumerate(DCH):
                    nc.sync.dma_start(xt_tiles[k][:, t0:t1, :],
                                      xt_d[c0:c1, t0:t1, :])

            # ---- PSUM per chain, split per ACT reader ----
            Pif, Pg, Po = [], [], []
            for cb in range(NCB):
                Pif.append(pspool.tile([128, 2, 512], F32, name=f"pif{cb}",
                                       tag=f"pif{cb}"))
                Pg.append(pspool.tile([128, 512], F32, name=f"pg{cb}",
                                      tag=f"pg{cb}"))
                Po.append(pspool.tile([128, 512], F32, name=f"po{cb}",
                                      tag=f"po{cb}"))

            def emit_einsum_bcast(j, y_half, odram, l0, nl, engine):
                """broadcast tensor_mul form (good on GpSimd)."""
                rows = j * 128
                ol = pool.tile([128, nl, M3, 128], F16, name="ol",
                               tag="outl", bufs=10)
                y_b = y_half[:, None, None, :].broadcast_to(
                    [128, nl, M3, 128])
                p_b = pf_t[:, j, l0:l0 + nl, :, None].broadcast_to(
                    [128, nl, M3, 128])
                engine.tensor_mul(ol[:], y_b, p_b)
                nc.gpsimd.dma_start(
                    odram[rows:rows + 128, l0:l0 + nl, :, :], ol[:])

            def emit_einsum_ts(j, y_half, odram, l0, nl, engine, use_act):
                """per-(l,m) form: DVE tensor_scalar (2x) or ACT mul."""
                rows = j * 128
                ol = pool.tile([128, nl, M3, 128], F16, name="ol",
                               tag="outl", bufs=10)
                for li in range(nl):
                    for m in range(M3):
                        sc = pf32_t[:, j, l0 + li, m:m + 1]
                        if use_act:
                            engine.mul(ol[:, li, m, :], y_half[:], sc)
                        else:
                            engine.tensor_scalar_mul(ol[:, li, m, :],
                                                     y_half[:], sc)
                nc.gpsimd.dma_start(
                    odram[rows:rows + 128, l0:l0 + nl, :, :], ol[:])

            store_jobs = []          # deferred y2 einsum+store emitters

            # ---- y2 head (prologue): borrows Pif[0] slices before t=0 ----
            for j in range(NJ):
                tgt = Pif[0][:, j // 4, (j % 4) * 128:(j % 4) * 128 + 128]
                nc.tensor.matmul(tgt, cnt_t[:, j * 128:(j + 1) * 128],
                                 fcw_t[:], start=True, stop=False)
                nc.tensor.matmul(tgt, ones_t[:], fcb_t[:],
                                 start=False, stop=True)
                yb = pool.tile([128, 128], F16, name="y2b", tag="y2b",
                               bufs=NJ)
                nc.scalar.activation(yb[:], tgt, AF.Relu)
                if j < NJ - 1:          # DVE tensor_scalar, 3l/2l bites
                    for l0, nl in ((0, 3), (3, 3), (6, 2), (8, 2)):
                        store_jobs.append(('ts', j, yb, oy2, l0, nl))
                else:                   # last j-tile: ACT in 1l bites
                    for l0 in range(L):
                        store_jobs.append(('act', j, yb, oy2, l0, 1))

            def emit_proj(t, cb, stop):
                cs = slice(cb * CW, (cb + 1) * CW)

                def mm(dst, g, k):
                    nc.tensor.matmul(
                        dst, wih_t[k][:, g * 128:(g + 1) * 128],
                        xt_tiles[k][:, t, cs],
                        start=(k == 0), stop=(stop and k == 2))
                for k in range(3):              # i, f pairs first
                    mm(Pif[cb][:, 0, 0:CW], 0, k)
                    mm(Pif[cb][:, 1, 0:CW], 1, k)
                for k in range(3):              # then g
                    mm(Pg[cb][:, 0:CW], 2, k)
                for k in range(3):              # then o
                    mm(Po[cb][:, 0:CW], 3, k)

            def emit_rec(cb, h_prev):
                for g, dst in ((0, Pif[cb][:, 0, 0:CW]),
                               (1, Pif[cb][:, 1, 0:CW]),
                               (2, Pg[cb][:, 0:CW]),
                               (3, Po[cb][:, 0:CW])):
                    nc.tensor.matmul(dst, whh_t[:, g * 128:(g + 1) * 128],
                                     h_prev[:], start=False, stop=True)

            def new_state(tag):
                return pool.tile([128, CW], F16, name=tag, tag=tag, bufs=2)

            h_prev = [None, None]
            c_prev = [None, None]

            # ---- prologue projections for t=0 ----
            emit_proj(0, 0, stop=True)
            emit_proj(0, 1, stop=True)

            # ---- recurrence ----
            job_i = 0
            for t in range(T):
                if t > 0:
                    emit_rec(0, h_prev[0])
                    emit_rec(1, h_prev[1])

                gif = [pool.tile([128, 2, CW], F16, name="gif",
                                 tag=f"gif{cb}", bufs=2) for cb in range(NCB)]
                gg = [new_state(f"gg{cb}") for cb in range(NCB)]
                go = [new_state(f"go{cb}") for cb in range(NCB)]
                c_t = ([new_state(f"c{cb}") for cb in range(NCB)]
                       if t > 0 else [None, None])
                tc_t = [new_state(f"tc{cb}") for cb in range(NCB)]
                h_t = [new_state(f"h{cb}") for cb in range(NCB)]
                t1 = ([new_state(f"t1{cb}") for cb in range(NCB)]
                      if t > 0 else [None, None])
                t2 = [new_state(f"t2{cb}") for cb in range(NCB)]

                for cb in range(NCB):
                    nc.scalar.activation(gif[cb][:], Pif[cb][:, :, 0:CW],
                                         AF.Sigmoid)
                    nc.scalar.activation(gg[cb][:], Pg[cb][:, 0:CW], AF.Tanh)
                    nc.scalar.activation(go[cb][:], Po[cb][:, 0:CW],
                                         AF.Sigmoid)
                    if t > 0:
                        nc.vector.tensor_mul(t1[cb][:], gif[cb][:, 1, :],
                                             c_prev[cb][:])
                    nc.vector.tensor_mul(t2[cb][:], gif[cb][:, 0, :],
                                         gg[cb][:])
                    if t > 0:
                        nc.vector.tensor_add(c_t[cb][:], t1[cb][:],
                                             t2[cb][:])
                    else:
                        c_t[cb] = t2[cb]
                # tanh(c) + h after both chains' gate ACTs are queued
                for cb in range(NCB):
                    nc.scalar.activation(tc_t[cb][:], c_t[cb][:], AF.Tanh)
                    nc.vector.tensor_mul(h_t[cb][:], go[cb][:], tc_t[cb][:])

                # PE: projections for t+1
                if t + 1 < T:
                    emit_proj(t + 1, 0, stop=False)
                    emit_proj(t + 1, 1, stop=False)

                h_prev = h_t
                c_prev = c_t

                # y2 einsum+stores through the recurrence.  DVE
                # tensor_scalar rides the dedicated SBUF port; GpSimd bites
                # are kept <=2l so their exclusive hold of the shared
                # DVE/GpSimd port pair can't delay the h-chain by much.
                if t >= 2:
                    if t < T - 1:
                        take, n_ts, seen_act = [], 0, False
                        for job in store_jobs:
                            if job[0] == 'ts' and n_ts < 2:
                                take.append(job); n_ts += 1
                            elif job[0] == 'act' and not seen_act:
                                take.append(job); seen_act = True
                        for job in take:
                            store_jobs.remove(job)
                            kind, j, yb, od, l0, nl = job
                            if kind == 'ts':
                                emit_einsum_ts(j, yb, od, l0, nl,
                                               nc.vector, False)
                            else:
                                emit_einsum_ts(j, yb, od, l0, nl,
                                               nc.scalar, True)
                    else:
                        for job in store_jobs:
                            kind, j, yb, od, l0, nl = job
                            eng = nc.vector if kind == 'ts' else nc.scalar
                            emit_einsum_ts(j, yb, od, l0, nl, eng,
                                           kind != 'ts')
                        store_jobs.clear()

            # ---- tail: y1 half ----
            NJH = NJ // 2
            y1b = []
            for j in range(NJ):
                y1 = pool.tile([128, 128], F16, name="y1b", tag="y1b",
                               bufs=NJ)
                src = h_prev[j // NJH][:, (j % NJH) * 128:(j % NJH) * 128 + 128]
                nc.sync.dma_start_transpose(y1[:], src)
                y1b.append(y1)
            # split 16 (j, l-chunk) jobs: gpsimd 6, DVE 6, ACT 4
            jobs = [(j, l0) for j in range(NJ) for l0 in range(0, L, 5)]
            for idx, (j, l0) in enumerate(jobs):
                r = idx % 16
                if r in (0, 2, 5, 7, 10, 12, 14):
                    emit_einsum_bcast(j, y1b[j], oy1, l0, 5, nc.gpsimd)
                elif r in (1, 6, 11, 15):
                    emit_einsum_ts(j, y1b[j], oy1, l0, 5, nc.scalar, True)
                else:
                    emit_einsum_ts(j, y1b[j], oy1, l0, 5, nc.vector, False)

    nc.compile()
    return nc


def prep_in_maps(inputs, n_cores: int, b_loc: int):
    history = np.asarray(inputs["history"], np.float32)
    cluster = np.asarray(inputs["cluster_num"], np.float32)
    pref = np.asarray(inputs["pref"], np.float32)
    w_ih = np.asarray(inputs["W_ih"], np.float32)
    w_hh = np.asarray(inputs["W_hh"], np.float32)
    b_ih = np.asarray(inputs["b_ih"], np.float32)
    b_hh = np.asarray(inputs["b_hh"], np.float32)
    fc_w = np.asarray(inputs["fc_w"], np.float32)
    fc_b = np.asarray(inputs["fc_b"], np.float32)

    NJ = b_loc // 128
    w_ih4 = np.concatenate(
        [w_ih.T, (b_ih + b_hh).reshape(1, 4 * H)], axis=0)  # [341, 512]
    shared = {
        "w_ih4": np.ascontiguousarray(w_ih4.astype(np.float16)),
        "w_hh_t": np.ascontiguousarray(w_hh.T.astype(np.float16)),
        "fc_w_t": np.ascontiguousarray(fc_w.T.astype(np.float16)),
        "fc_b_row": np.ascontiguousarray(fc_b.reshape(1, H).astype(np.float16)),
        "ones_row": np.ones((1, 128), np.float16),
    }
    in_maps = []
    for c in range(n_cores):
        r0, r1 = c * b_loc, (c + 1) * b_loc
        hist16 = history[r0:r1].reshape(b_loc, T, D).astype(np.float16)
        xt = np.empty((DP, T, b_loc), np.float16)
        xt[:D] = hist16.transpose(2, 1, 0)
        xt[D] = 1.0
        pref16 = pref[r0:r1].reshape(NJ, 128, L, M3).astype(np.float16)
        pg = np.ascontiguousarray(pref16.transpose(1, 0, 2, 3))
        in_maps.append({
            "xt": xt,
            "cn_t": np.ascontiguousarray(
                cluster[r0:r1].T.astype(np.float16)),
            "pref_g": pg,
            "pref_g32": pg.astype(np.float32),
            **shared,
        })
    return in_maps


def run(inputs, n_cores: int = N_CORES, trace: bool = False):
    B = np.asarray(inputs["history"]).shape[0]
    b_loc = B // n_cores
    nc = build_program(b_loc)
    in_maps = prep_in_maps(inputs, n_cores, b_loc)
    res = bass_utils.run_bass_kernel_spmd(
        nc, in_maps, core_ids=list(range(n_cores)), trace=trace)
    outs = []
    for c in range(n_cores):
        y1 = res.results[c]["out_y1"].astype(np.float32)
        y2 = res.results[c]["out_y2"].astype(np.float32)
        o = np.concatenate([y1, y2], axis=3)         # [b, L, M3, 256]
        outs.append(o.transpose(0, 1, 3, 2).reshape(b_loc, L, 256 * M3))
    return np.concatenate(outs, axis=0), res


def kernel(**inputs) -> np.ndarray:
    out, _ = run(inputs, N_CORES)
    return out


# revision 21
# speedup vs baseline: 1.0264x; 1.0155x over previous
"""Trainium2 Bass kernel for nn_BackBone (LSTM backbone + fc + outer-product head).

Data-parallel over batch across 8 NeuronCores. Per core (b_loc rows), v4:
  - history transposed + cast to fp16 on the HOST: xt[D+1, T, B] with a
    constant-1.0 feature row appended so the gate bias rides the projection
    matmul (no PE transposes, no SWDGE cast, half the input HBM traffic)
  - two 512-col batch chains. Tile WAR deps are TILE-granular, so the gate
    PSUM is split per READER: Pif [128,2,512] (read only by the merged
    sigmoid(i,f)), Pg (tanh), Po (sigmoid) — a projection matmul for gate X
    then only waits on gate X's own activation from the previous step.
  - PE stream per step: recA(i,f,g,o), recB, projA(t+1) in reader order
    (if x3k, g x3k, o x3k), projB(t+1) — zero-stall steady state
  - head einsum m-major (out[b, l, m, n]; host un-permutes): y2 half on
    GpSimd (idle engine; broadcast muls run FASTER there than on DVE),
    y1 tail split GpSimd (broadcast) + DVE (per-(l,m) tensor_scalar, 2x
    mode) + ACT (per-(l,m) scalar.mul)
  - output fp16 as two [b, L, 3, 128] tensors; host concatenates/permutes/
    casts; loads on sync HWDGE, stores on gpsimd SWDGE
"""
import numpy as np

import concourse.bacc as bacc
import concourse.mybir as mybir
import concourse.tile as tile
from concourse import bass_utils

F32 = mybir.dt.float32
F16 = mybir.dt.float16
AF = mybir.ActivationFunctionType

T = 20
D = 340
DP = D + 1               # +1 constant feature row carrying the gate bias
H = 128
E = 32
L = 10
M3 = 3
DCH = [(0, 128), (128, 256), (256, DP)]   # contraction chunks of DP
N_CORES = 8


def build_program(b_loc: int):
    assert b_loc % 256 == 0
    NJ = b_loc // 128
    CW = b_loc // 2               # chain width (<= 512)
    assert CW <= 512
    NCB = 2
    if T == 20:
        TGR = [(0, 1), (1, 2)] + [(t, t + 2) for t in range(2, 14, 2)] \
            + [(14, 17), (17, 20)]
    else:
        TGR = [(0, T)]

    nc = bacc.Bacc("TRN2", target_bir_lowering=False, debug=False)
    xt_d = nc.dram_tensor("xt", (DP, T, b_loc), F16, kind="ExternalInput").ap()
    cnt_d = nc.dram_tensor("cn_t", (E, b_loc), F16, kind="ExternalInput").ap()
    pref_d = nc.dram_tensor("pref_g", (128, NJ, L, M3), F16,
                            kind="ExternalInput").ap()
    pref32_d = nc.dram_tensor("pref_g32", (128, NJ, L, M3), F32,
                              kind="ExternalInput").ap()
    wih_d = nc.dram_tensor("w_ih4", (DP, 4 * H), F16, kind="ExternalInput").ap()
    whh_d = nc.dram_tensor("w_hh_t", (H, 4 * H), F16, kind="ExternalInput").ap()
    fcw_d = nc.dram_tensor("fc_w_t", (E, H), F16, kind="ExternalInput").ap()
    fcb_d = nc.dram_tensor("fc_b_row", (1, H), F16, kind="ExternalInput").ap()
    ones_d = nc.dram_tensor("ones_row", (1, 128), F16, kind="ExternalInput").ap()
    oy1 = nc.dram_tensor("out_y1", (b_loc, L, M3, 128), F16,
                         kind="ExternalOutput").ap()
    oy2 = nc.dram_tensor("out_y2", (b_loc, L, M3, 128), F16,
                         kind="ExternalOutput").ap()

    with tile.TileContext(nc) as tc:
        with tc.tile_pool(name="wpool", bufs=1) as wpool, \
             tc.tile_pool(name="main", bufs=1) as pool, \
             tc.tile_pool(name="psum", bufs=1, space="PSUM") as pspool:

            # ---- weights / constants ----
            wih_t = []
            for k, (c0, c1) in enumerate(DCH):
                wt_ = wpool.tile([c1 - c0, 4 * H], F16, name=f"wih{k}")
                nc.sync.dma_start(wt_[:], wih_d[c0:c1, :])
                wih_t.append(wt_)
            whh_t = wpool.tile([H, 4 * H], F16, name="whh_t")
            nc.sync.dma_start(whh_t[:], whh_d)
            cnt_t = wpool.tile([E, b_loc], F16, name="cnt_t")
            nc.sync.dma_start(cnt_t[:], cnt_d)
            fcw_t = wpool.tile([E, H], F16, name="fcw_t")
            nc.sync.dma_start(fcw_t[:], fcw_d)
            fcb_t = wpool.tile([1, H], F16, name="fcb_t")
            nc.sync.dma_start(fcb_t[:], fcb_d)
            ones_t = wpool.tile([1, 128], F16, name="ones_t")
            nc.sync.dma_start(ones_t[:], ones_d)
            pf_t = wpool.tile([128, NJ, L, M3], F16, name="pf_t")
            nc.sync.dma_start(pf_t[:], pref_d)
            pf32_t = wpool.tile([128, NJ, L, M3], F32, name="pf32_t")
            nc.sync.dma_start(pf32_t[:], pref32_d)

            # ---- persistent fp16 xT tiles, loaded in t-groups ----
            xt_tiles = []
            for k, (c0, c1) in enumerate(DCH):
                xt_tiles.append(
                    pool.tile([c1 - c0, T, b_loc], F16, name=f"xt{k}",
                              tag=f"xt{k}"))
            for (t0, t1) in TGR:
                for k, (c0, c1) in enumerate(DCH):
                    nc.sync.dma_start(xt_tiles[k][:, t0:t1, :],
                                      xt_d[c0:c1, t0:t1, :])

            # ---- PSUM per chain, split per ACT reader ----
            Pif, Pg, Po = [], [], []
            for cb in range(NCB):
                Pif.append(pspool.tile([128, 2, 512], F32, name=f"pif{cb}",
                                       tag=f"pif{cb}"))
                Pg.append(pspool.tile([128, 512], F32, name=f"pg{cb}",
                                      tag=f"pg{cb}"))
                Po.append(pspool.tile([128, 512], F32, name=f"po{cb}",
                                      tag=f"po{cb}"))

            def emit_einsum_bcast(j, y_half, odram, l0, nl, engine,
                                  store_eng=None):
                """broadcast tensor_mul form (good on GpSimd)."""
                rows = j * 128
                ol = pool.tile([128, nl, M3, 128], F16, name="ol",
                               tag="outl", bufs=10)
                y_b = y_half[:, None, None, :].broadcast_to(
                    [128, nl, M3, 128])
                p_b = pf_t[:, j, l0:l0 + nl, :, None].broadcast_to(
                    [128, nl, M3, 128])
                engine.tensor_mul(ol[:], y_b, p_b)
                (store_eng or nc.gpsimd).dma_start(
                    odram[rows:rows + 128, l0:l0 + nl, :, :], ol[:])

            def emit_einsum_ts(j, y_half, odram, l0, nl, engine, use_act,
                               store_eng=None):
                """per-(l,m) form: DVE tensor_scalar (2x) or ACT mul."""
                rows = j * 128
                ol = pool.tile([128, nl, M3, 128], F16, name="ol",
                               tag="outl", bufs=10)
                for li in range(nl):
                    for m in range(M3):
                        sc = pf32_t[:, j, l0 + li, m:m + 1]
                        if use_act:
                            engine.mul(ol[:, li, m, :], y_half[:], sc)
                        else:
                            engine.tensor_scalar_mul(ol[:, li, m, :],
                                                     y_half[:], sc)
                (store_eng or nc.gpsimd).dma_start(
                    odram[rows:rows + 128, l0:l0 + nl, :, :], ol[:])

            store_jobs = []          # deferred y2 einsum+store emitters

            # ---- y2 head (prologue): borrows Pif[0] slices before t=0 ----
            for j in range(NJ):
                tgt = Pif[0][:, j // 4, (j % 4) * 128:(j % 4) * 128 + 128]
                nc.tensor.matmul(tgt, cnt_t[:, j * 128:(j + 1) * 128],
                                 fcw_t[:], start=True, stop=False)
                nc.tensor.matmul(tgt, ones_t[:], fcb_t[:],
                                 start=False, stop=True)
                yb = pool.tile([128, 128], F16, name="y2b", tag="y2b",
                               bufs=NJ)
                nc.scalar.activation(yb[:], tgt, AF.Relu)
                if j < NJ - 1:          # DVE tensor_scalar, 3l/2l bites
                    for l0, nl in ((0, 3), (3, 3), (6, 2), (8, 2)):
                        store_jobs.append(('ts', j, yb, oy2, l0, nl))
                else:                   # last j-tile: ACT in 1l bites
                    for l0 in range(L):
                        store_jobs.append(('act', j, yb, oy2, l0, 1))

            def emit_proj(t, cb, stop):
                cs = slice(cb * CW, (cb + 1) * CW)

                def mm(dst, g, k):
                    nc.tensor.matmul(
                        dst, wih_t[k][:, g * 128:(g + 1) * 128],
                        xt_tiles[k][:, t, cs],
                        start=(k == 0), stop=(stop and k == 2))
                for k in range(3):              # i, f pairs first
                    mm(Pif[cb][:, 0, 0:CW], 0, k)
                    mm(Pif[cb][:, 1, 0:CW], 1, k)
                for k in range(3):              # then g
                    mm(Pg[cb][:, 0:CW], 2, k)
                for k in range(3):              # then o
                    mm(Po[cb][:, 0:CW], 3, k)

            def emit_rec(cb, h_prev):
                for g, dst in ((0, Pif[cb][:, 0, 0:CW]),
                               (1, Pif[cb][:, 1, 0:CW]),
                               (2, Pg[cb][:, 0:CW]),
                               (3, Po[cb][:, 0:CW])):
                    nc.tensor.matmul(dst, whh_t[:, g * 128:(g + 1) * 128],
                                     h_prev[:], start=False, stop=True)

            def new_state(tag):
                return pool.tile([128, CW], F16, name=tag, tag=tag, bufs=2)

            h_prev = [None, None]
            c_prev = [None, None]

            # ---- prologue projections for t=0 ----
            emit_proj(0, 0, stop=True)
            emit_proj(0, 1, stop=True)

            # ---- recurrence ----
            job_i = 0
            for t in range(T):
                if t > 0:
                    emit_rec(0, h_prev[0])
                    emit_rec(1, h_prev[1])

                gif = [pool.tile([128, 2, CW], F16, name="gif",
                                 tag=f"gif{cb}", bufs=2) for cb in range(NCB)]
                gg = [new_state(f"gg{cb}") for cb in range(NCB)]
                go = [new_state(f"go{cb}") for cb in range(NCB)]
                c_t = ([new_state(f"c{cb}") for cb in range(NCB)]
                       if t > 0 else [None, None])
                tc_t = [new_state(f"tc{cb}") for cb in range(NCB)]
                h_t = [new_state(f"h{cb}") for cb in range(NCB)]
                t1 = ([new_state(f"t1{cb}") for cb in range(NCB)]
                      if t > 0 else [None, None])
                t2 = [new_state(f"t2{cb}") for cb in range(NCB)]

                for cb in range(NCB):
                    nc.scalar.activation(gif[cb][:], Pif[cb][:, :, 0:CW],
                                         AF.Sigmoid)
                    nc.scalar.activation(gg[cb][:], Pg[cb][:, 0:CW], AF.Tanh)
                    nc.scalar.activation(go[cb][:], Po[cb][:, 0:CW],
                                         AF.Sigmoid)
                    if t > 0:
                        nc.vector.tensor_mul(t1[cb][:], gif[cb][:, 1, :],
                                             c_prev[cb][:])
                    nc.vector.tensor_mul(t2[cb][:], gif[cb][:, 0, :],
                                         gg[cb][:])
                    if t > 0:
                        nc.vector.tensor_add(c_t[cb][:], t1[cb][:],
                                             t2[cb][:])
                    else:
                        c_t[cb] = t2[cb]
                # tanh(c) + h after both chains' gate ACTs are queued
                for cb in range(NCB):
                    nc.scalar.activation(tc_t[cb][:], c_t[cb][:], AF.Tanh)
                    nc.vector.tensor_mul(h_t[cb][:], go[cb][:], tc_t[cb][:])

                # PE: projections for t+1
                if t + 1 < T:
                    emit_proj(t + 1, 0, stop=False)
                    emit_proj(t + 1, 1, stop=False)

                h_prev = h_t
                c_prev = c_t

                # y2 einsum+stores through the recurrence.  DVE
                # tensor_scalar rides the dedicated SBUF port; GpSimd bites
                # are kept <=2l so their exclusive hold of the shared
                # DVE/GpSimd port pair can't delay the h-chain by much.
                if t >= 2:
                    if t < T - 1:
                        take, n_ts, seen_act = [], 0, False
                        for job in store_jobs:
                            if job[0] == 'ts' and n_ts < 2:
                                take.append(job); n_ts += 1
                            elif job[0] == 'act' and not seen_act:
                                take.append(job); seen_act = True
                        for job in take:
                            store_jobs.remove(job)
                            kind, j, yb, od, l0, nl = job
                            if kind == 'ts':
                                emit_einsum_ts(j, yb, od, l0, nl,
                                               nc.vector, False)
                            else:
                                emit_einsum_ts(j, yb, od, l0, nl,
                                               nc.scalar, True)
                    else:
                        for job in store_jobs:
                            kind, j, yb, od, l0, nl = job
                            eng = nc.vector if kind == 'ts' else nc.scalar
                            emit_einsum_ts(j, yb, od, l0, nl, eng,
                                           kind != 'ts')
                        store_jobs.clear()

            # ---- tail: y1 half ----
            NJH = NJ // 2
            y1b = []
            for j in range(NJ):
                y1 = pool.tile([128, 128], F16, name="y1b", tag="y1b",
                               bufs=NJ)
                src = h_prev[j // NJH][:, (j % NJH) * 128:(j % NJH) * 128 + 128]
                nc.sync.dma_start_transpose(y1[:], src)
                y1b.append(y1)
            # split 16 (j, l-chunk) jobs: gpsimd 6, DVE 6, ACT 4
            jobs = [(j, l0) for j in range(NJ) for l0 in range(0, L, 5)]
            for idx, (j, l0) in enumerate(jobs):
                r = idx % 16
                if r in (0, 2, 5, 7, 10, 12, 14):
                    emit_einsum_bcast(j, y1b[j], oy1, l0, 5, nc.gpsimd,
                                      store_eng=nc.sync)
                elif r in (1, 6, 11, 15):
                    emit_einsum_ts(j, y1b[j], oy1, l0, 5, nc.scalar, True,
                                   store_eng=nc.sync)
                else:
                    emit_einsum_ts(j, y1b[j], oy1, l0, 5, nc.vector, False,
                                   store_eng=nc.sync)

    nc.compile()
    return nc


def prep_in_maps(inputs, n_cores: int, b_loc: int):
    history = np.asarray(inputs["history"], np.float32)
    cluster = np.asarray(inputs["cluster_num"], np.float32)
    pref = np.asarray(inputs["pref"], np.float32)
    w_ih = np.asarray(inputs["W_ih"], np.float32)
    w_hh = np.asarray(inputs["W_hh"], np.float32)
    b_ih = np.asarray(inputs["b_ih"], np.float32)
    b_hh = np.asarray(inputs["b_hh"], np.float32)
    fc_w = np.asarray(inputs["fc_w"], np.float32)
    fc_b = np.asarray(inputs["fc_b"], np.float32)

    NJ = b_loc // 128
    w_ih4 = np.concatenate(
        [w_ih.T, (b_ih + b_hh).reshape(1, 4 * H)], axis=0)  # [341, 512]
    shared = {
        "w_ih4": np.ascontiguousarray(w_ih4.astype(np.float16)),
        "w_hh_t": np.ascontiguousarray(w_hh.T.astype(np.float16)),
        "fc_w_t": np.ascontiguousarray(fc_w.T.astype(np.float16)),
        "fc_b_row": np.ascontiguousarray(fc_b.reshape(1, H).astype(np.float16)),
        "ones_row": np.ones((1, 128), np.float16),
    }
    in_maps = []
    for c in range(n_cores):
        r0, r1 = c * b_loc, (c + 1) * b_loc
        hist16 = history[r0:r1].reshape(b_loc, T, D).astype(np.float16)
        xt = np.empty((DP, T, b_loc), np.float16)
        xt[:D] = hist16.transpose(2, 1, 0)
        xt[D] = 1.0
        pref16 = pref[r0:r1].reshape(NJ, 128, L, M3).astype(np.float16)
        pg = np.ascontiguousarray(pref16.transpose(1, 0, 2, 3))
        in_maps.append({
            "xt": xt,
            "cn_t": np.ascontiguousarray(
                cluster[r0:r1].T.astype(np.float16)),
            "pref_g": pg,
            "pref_g32": pg.astype(np.float32),
            **shared,
        })
    return in_maps


def run(inputs, n_cores: int = N_CORES, trace: bool = False):
    B = np.asarray(inputs["history"]).shape[0]
    b_loc = B // n_cores
    nc = build_program(b_loc)
    in_maps = prep_in_maps(inputs, n_cores, b_loc)
    res = bass_utils.run_bass_kernel_spmd(
        nc, in_maps, core_ids=list(range(n_cores)), trace=trace)
    outs = []
    for c in range(n_cores):
        y1 = res.results[c]["out_y1"].astype(np.float32)
        y2 = res.results[c]["out_y2"].astype(np.float32)
        o = np.concatenate([y1, y2], axis=3)         # [b, L, M3, 256]
        outs.append(o.transpose(0, 1, 3, 2).reshape(b_loc, L, 256 * M3))
    return np.concatenate(outs, axis=0), res


def kernel(**inputs) -> np.ndarray:
    out, _ = run(inputs, N_CORES)
    return out
